# revision 46
# baseline (speedup 1.0000x reference)
"""Trainium2 Bass kernel for nn_Attention_64063732187236 (v4).

Reference computation (B=4, N=2048, DIM=512, HEADS=8, DIM_HEAD=64):
    qkv = x @ w_qkv ; q,k,v = split(qkv) -> [b,h,n,64]
    atten = softmax((q * HEADS**-0.5) @ k^T + drop_mask * -1e12)
    out   = (atten @ v) reshaped -> [b,n,512] @ w_out

Sharding: 8 cores = 4 batches x 2 head-groups (4 heads each); host sums the
two head-group partial outputs per batch.

v4 design (cost-model driven; ~110us/core PE floor):
- Transposed PV: out[q,d] psum tiles via lhsT=pt[128keys,128q] slices and
  rhs=v[128,65] -> PV costs 65 cycles/matmul (27.7us total vs 54.6 in the
  [d,q] orientation), and the rowsum column lands per-partition so softmax
  normalization folds into the psum->SBUF copy as a per-partition
  broadcast multiply (free).
- The normalized pvq [128q, (qc, head-parity, d)] tile is transposed back
  to attnT [2*64 dims, qc, 128 q] by ONE DMA-engine xbar transpose per
  (pair, half) (out[p,t,c] = in[c, t*128+p]), so no PE transpose and no
  extra copies; out-projection reads attnT directly (128-contraction).
- exp split three ways to balance ACT/DVE/Pool below the PE floor:
  * ACT route: exact exp from psum, then mask multiply {0,1}:
    - on DVE (bf16 mask, 2x rate), or
    - on Pool/gpsimd (u8 mask -- 1-byte DMA, dtype-blind Q7 rate).
  * DVE fused route: tensor_tensor add of an fp8e5 additive mask
    {0, -1024} onto the psum scores (bf16 out), then a 4x-rate
    tensor_scalar Schraudolph exp (bitcast-i16); masked entries drive the
    i16 round to saturation = 0x8000 = -0.0 exactly.
- PV(g) runs one half behind scores(g) (query-chunk-outer so psum
  accumulation groups stay sequential per bank); g-order
  h0A h0B h1A h1B h2A h3A h2B h3B lets half-A out-projections ride g7.
- PSUM: psS 2x[128,1024]f32 (4 banks) + pvT 2x[128,4,2,64] (2) +
  rs [128,512] (1) + po [128,512] (1) = 8 banks exactly.
"""

import numpy as np

import concourse.bass as bass
import concourse.bacc as bacc
import concourse.tile as tile
from concourse import mybir
from concourse.bass_utils import run_bass_kernel_spmd

F32 = mybir.dt.float32
BF16 = mybir.dt.bfloat16
U8 = mybir.dt.uint8
FP8E5 = mybir.dt.float8e5
I16 = mybir.dt.int16
NP_BF16 = mybir.dt.np(BF16)
NP_FP8E5 = mybir.dt.np(FP8E5)

# Full-size problem constants
B, N, D = 4, 2048, 512
HEADS, DH = 8, 64
HL = 4                # heads per core (local)
GROUPS = HEADS // HL  # head groups = 2
HP = HL // 2          # head pairs per core
KT = N // 128         # key tiles
HALF = N // 2
SCALE = float(HEADS) ** -0.5   # reference quirk: scales by heads, not dim_head
Exp = mybir.ActivationFunctionType.Exp
A16 = 128.0 / float(np.log(2.0))
B16 = 127.0 * 128 - 8
MADD = -1024.0  # additive mask value (exact in fp8e5); saturates TS to -0.0

# g order: (h, half) so that both heads of a pair finish PV close together
# and half-A out-projections can ride g7.
GORDER = [(0, 0), (0, 1), (1, 0), (1, 1), (2, 0), (3, 0), (2, 1), (3, 1)]

# Route per (h, kt) unit, applied to both halves:
#   'A' = ACT exp + DVE bf16 mask-mul
#   'P' = ACT exp + Pool u8 mask-mul
#   'F' = DVE fused fp8e5 additive mask + 4x Schraudolph
# Per head: 16 kt -> mix tuned so ACT ~100 tiles, DVE-fused 28, Pool 48.
_PAT = ['P', 'F', 'P', 'A', 'P', 'F', 'P', 'A', 'P', 'F', 'P', 'A', 'F', 'A', 'A', 'A']
_PAT3 = ['P', 'F', 'P', 'A', 'P', 'F', 'P', 'A', 'P', 'F', 'P', 'A', 'F', 'A', 'A', 'A']
ROUTE = {h: list(_PAT if h < 3 else _PAT3) for h in range(HL)}
# index of each (h, kt) unit within its dtype-packed host tensor
_IDX = {'A': {}, 'P': {}, 'F': {}}
for _h in range(HL):
    for _kt in range(KT):
        r = ROUTE[_h][_kt]
        _IDX[r][(_h, _kt)] = len(_IDX[r])
NA, NP_, NF = len(_IDX['A']), len(_IDX['P']), len(_IDX['F'])


def build_graph(nc):
    tc = nc.tc

    xT = nc.dram_tensor("xT", [D, N], BF16, kind="ExternalInput").ap()
    wqkv = nc.dram_tensor("wqkv", [D, 3 * HL * DH], BF16, kind="ExternalInput").ap()
    wout = nc.dram_tensor("wout", [HL * DH, D], BF16, kind="ExternalInput").ap()
    # A-route masks packed per half: [unit, half, 128 keys, 1024 queries]
    nmb = nc.dram_tensor("nmb", [max(NA, 1), 2, 128, HALF], BF16,
                         kind="ExternalInput").ap()
    nmu = nc.dram_tensor("nmu", [max(NP_, 1), 128, N], U8,
                         kind="ExternalInput").ap()
    nmf = nc.dram_tensor("nmf", [max(NF, 1), 128, N], FP8E5,
                         kind="ExternalInput").ap()
    out = nc.dram_tensor("out", [N, D], BF16, kind="ExternalOutput").ap()

    with tc.tile_pool(name="wts", bufs=1) as wts, \
         tc.tile_pool(name="persist", bufs=1) as persist, \
         tc.tile_pool(name="pmb", bufs=14) as pmb, \
         tc.tile_pool(name="pmu", bufs=12) as pmu, \
         tc.tile_pool(name="pmf", bufs=8) as pmf, \
         tc.tile_pool(name="pprob", bufs=33) as pprob, \
         tc.tile_pool(name="psarg", bufs=2) as psarg, \
         tc.tile_pool(name="pout", bufs=4) as pout:

        # ---- weights / inputs ----
        wb = [wts.tile([128, 3 * HL * DH], BF16, tag=f"wb{dc}", name=f"wb{dc}")
              for dc in range(4)]
        xTb = [wts.tile([128, N], BF16, tag=f"xtb{dc}", name=f"xtb{dc}")
               for dc in range(4)]
        woutb = [wts.tile([128, D], BF16, tag=f"wob{c}", name=f"wob{c}")
                 for c in range(HP)]
        wdum = wts.tile([128, 512], BF16, tag="wdum", name="wdum")
        nc.vector.memset(wdum, 0.0)

        for dc in range(4):
            nc.sync.dma_start(out=wb[dc], in_=wqkv[dc * 128:(dc + 1) * 128, :])
            nc.sync.dma_start(out=xTb[dc][:, 0:512],
                              in_=xT[dc * 128:(dc + 1) * 128, 0:512])
            nc.scalar.dma_start(out=xTb[dc][:, 512:HALF],
                                in_=xT[dc * 128:(dc + 1) * 128, 512:HALF])
        for c in range(HP):
            nc.sync.dma_start(out=woutb[c], in_=wout[c * 128:(c + 1) * 128, :])
        for dc in range(4):
            nc.scalar.dma_start(out=xTb[dc][:, HALF:N],
                                in_=xT[dc * 128:(dc + 1) * 128, HALF:N])

        # ---- persistent tiles ----
        qTb = [persist.tile([128, N], BF16, tag=f"qT{p}", name=f"qT{p}")
               for p in range(HP)]
        kTb = [persist.tile([128, N], BF16, tag=f"kT{p}", name=f"kT{p}")
               for p in range(HP)]
        vplus = [persist.tile([128, HL, DH + 1], BF16, tag=f"vp{t}",
                              name=f"vp{t}") for t in range(KT)]
        # pvq[(hp, half)]: [128 q, qc, head-parity, d] normalized bf16
        pvq = {}
        attnT = {}
        rinv = {}
        for hp in range(HP):
            for half in range(2):
                pvq[(hp, half)] = persist.tile(
                    [128, 8, 2, DH], BF16, tag=f"pvq{hp}{half}",
                    name=f"pvq{hp}{half}")
                attnT[(hp, half)] = persist.tile(
                    [128, 8, 128], BF16, tag=f"at{hp}{half}",
                    name=f"at{hp}{half}")
                rinv[(hp, half)] = persist.tile(
                    [128, 8, 2], F32, tag=f"ri{hp}{half}", name=f"ri{hp}{half}")

        # mask tiles per (h, kt) unit (P/F routes) or per (h, kt, half) (A)
        mtiles = {}

        def fetch_mask(h, kt, half=None):
            r = ROUTE[h][kt]
            if r == 'A':
                t = pmb.tile([128, HALF], BF16, tag="nmA", name=f"ma{h}_{kt}_{half}")
                nc.sync.dma_start(out=t, in_=nmb[_IDX['A'][(h, kt)], half])
                mtiles[(h, kt, half)] = t
            elif r == 'P':
                t = pmu.tile([128, N], U8, tag="nmP", name=f"mp{h}_{kt}")
                nc.sync.dma_start(out=t, in_=nmu[_IDX['P'][(h, kt)]])
                mtiles[(h, kt)] = t
            else:
                t = pmf.tile([128, N], FP8E5, tag="nmF", name=f"mf{h}_{kt}")
                nc.sync.dma_start(out=t, in_=nmf[_IDX['F'][(h, kt)]])
                mtiles[(h, kt)] = t

        # pt tiles per (g, kt): [128 keys, 1024 queries] bf16
        pt_of = {}

        with tc.tile_pool(name="psS", bufs=3, space="PSUM") as psS, \
             tc.tile_pool(name="psPV", bufs=1, space="PSUM") as psPV, \
             tc.tile_pool(name="psRS", bufs=1, space="PSUM") as psRS:

            # ---- projection filler units ----
            def qk_unit(which, hp, half, s0, act_copy=False):
                def emit():
                    off = which * HL * DH
                    col = half * HALF + s0
                    ps = psS.tile([128, 1024], F32, tag="s", name="psqk")
                    for dc in range(4):
                        nc.tensor.matmul(
                            ps[:, 0:512],
                            lhsT=wb[dc][:, off + hp * 128: off + (hp + 1) * 128],
                            rhs=xTb[dc][:, col:col + 512],
                            start=(dc == 0), stop=(dc == 3))
                    dst = qTb if which == 0 else kTb
                    if act_copy:
                        nc.scalar.copy(dst[hp][:, col:col + 512], ps[:, 0:512])
                    else:
                        with tc.high_priority(offset=110):
                            nc.vector.tensor_copy(dst[hp][:, col:col + 512],
                                                  ps[:, 0:512])
                return emit

            def v_unit(nt):
                def emit():
                    voff = 2 * HL * DH
                    ps = psS.tile([128, 1024], F32, tag="s", name="psv")
                    for dc in range(4):
                        nc.tensor.matmul(
                            ps[:, 0:HL * DH],
                            lhsT=xTb[dc][:, nt * 128:(nt + 1) * 128],
                            rhs=wb[dc][:, voff: voff + HL * DH],
                            start=(dc == 0), stop=(dc == 3))
                    nc.vector.memset(vplus[nt], 1.0)
                    with tc.high_priority(offset=110):
                        nc.vector.tensor_copy(
                            vplus[nt][:, :, 0:DH],
                            ps[:, 0:HL * DH].rearrange("p (h d) -> p h d", h=HL))
                return emit

            # filler schedule: {g: {kt: [unit...]}}
            FILL = {g: {} for g in range(8)}

            def put(g, kt, unit):
                FILL[g].setdefault(kt, []).append(unit)

            # g0 fillers: v4..v15 (v0-3 upfront), qT[hp0] halfB
            for i in range(4, KT):
                put(0, i - 4, v_unit(i))
            put(0, 12, qk_unit(0, 0, 1, 0))
            put(0, 14, qk_unit(0, 0, 1, 512))
            # g1-g3 fillers: kT[hp1] (4 units), qT[hp1] halfA (2), halfB (2)
            put(1, 2, qk_unit(1, 1, 0, 0))
            put(1, 6, qk_unit(1, 1, 0, 512))
            put(1, 10, qk_unit(1, 1, 1, 0))
            put(1, 14, qk_unit(1, 1, 1, 512))
            put(2, 2, qk_unit(0, 1, 0, 0))
            put(2, 8, qk_unit(0, 1, 0, 512))
            put(3, 2, qk_unit(0, 1, 1, 0))
            put(3, 8, qk_unit(0, 1, 1, 512))

            # ---- PV one half behind: per g, 8 qc groups (2 quarters) ----
            def emit_pv_group(gi, qc):
                """Query-chunk group qc (0..7) of PV for g=GORDER[gi]."""
                h, half = GORDER[gi]
                hp, hpar = h // 2, h % 2
                q4, qc_l = qc // 4, qc % 4
                key = (gi, q4)
                if key not in emit_pv_group.tiles:
                    emit_pv_group.tiles[key] = psPV.tile(
                        [128, 4, 2, DH], F32, tag="pv", name=f"pv{gi}_{q4}")
                pvT = emit_pv_group.tiles[key]
                if (gi,) not in emit_pv_group.rs:
                    emit_pv_group.rs[(gi,)] = psRS.tile(
                        [128, 512], F32, tag="rs", name=f"rs{gi}")
                rs = emit_pv_group.rs[(gi,)]
                col = qc * 2 + hpar
                # P-route tiles (Pool-masked, latest to complete) last
                kts = ([kt for kt in range(KT) if ROUTE[h][kt] != 'P']
                       + [kt for kt in range(KT) if ROUTE[h][kt] == 'P'])
                for i, kt in enumerate(kts):
                    pt = pt_of[(gi, kt)]
                    lhsT = pt[:, qc * 128:(qc + 1) * 128]
                    nc.tensor.matmul(
                        pvT[:, qc_l, hpar, :], lhsT=lhsT,
                        rhs=vplus[kt][:, h, 0:DH],
                        start=(i == 0), stop=(i == KT - 1))
                    nc.tensor.matmul(
                        rs[:, col:col + 1], lhsT=lhsT,
                        rhs=vplus[kt][:, h, DH:DH + 1],
                        start=(i == 0), stop=(i == KT - 1))

            emit_pv_group.tiles = {}
            emit_pv_group.rs = {}

            def emit_finisher(gi, q4):
                """Normalize quarter q4 of g=GORDER[gi] into pvq."""
                h, half = GORDER[gi]
                hp, hpar = h // 2, h % 2
                pvT = emit_pv_group.tiles.pop((gi, q4))
                rs = emit_pv_group.rs[(gi,)]
                rs16 = rs[:, 0:16].rearrange("p (a b) -> p a b", a=8)
                ri = rinv[(hp, half)]
                with tc.high_priority(offset=140):
                    nc.vector.reciprocal(ri[:, q4 * 4:(q4 + 1) * 4, hpar],
                                         rs16[:, q4 * 4:(q4 + 1) * 4, hpar])
                    nc.vector.tensor_tensor(
                        pvq[(hp, half)][:, q4 * 4:(q4 + 1) * 4, hpar, :],
                        pvT[:, :, hpar, :],
                        ri[:, q4 * 4:(q4 + 1) * 4, hpar].unsqueeze(2)
                          .broadcast_to([128, 4, DH]),
                        mybir.AluOpType.mult)

            def emit_transpose(hp, half, q4=None):
                if q4 is None:
                    nc.sync.dma_start(
                        out=attnT[(hp, half)],
                        in_=pvq[(hp, half)].rearrange("p a b d -> p (a b d)"),
                        transpose=True)
                else:
                    nc.sync.dma_start(
                        out=attnT[(hp, half)][:, q4 * 4:(q4 + 1) * 4, :],
                        in_=pvq[(hp, half)][:, q4 * 4:(q4 + 1) * 4]
                        .rearrange("p a b d -> p (a b d)"),
                        transpose=True)

            def out_proj(nt, copy_eng):
                half, qcl = nt // 8, nt % 8
                po = psS.tile([128, 1024], F32, tag="s", name="po")[:, 0:D]
                for c in range(HP):
                    nc.tensor.matmul(
                        po, lhsT=attnT[(c, half)][:, qcl, :],
                        rhs=woutb[c], start=(c == 0), stop=(c == HP - 1))
                ob = pout.tile([128, D], BF16, tag="ob", name="ob")
                if copy_eng == 'act':
                    nc.scalar.copy(ob, po)
                else:
                    nc.vector.tensor_copy(ob, po)
                nc.sync.dma_start(out=out[nt * 128:(nt + 1) * 128, :], in_=ob)

            # transposes done per (hp, half) once both heads' finishers ran
            fin_count = {}

            def note_finished(gi):
                h, half = GORDER[gi]
                hp = h // 2
                fin_count[(hp, half)] = fin_count.get((hp, half), 0) + 1
                if fin_count[(hp, half)] == 2:
                    emit_transpose(hp, half)

            # ---- PE warm-up + upfront projections ----
            for _ in range(5):
                wps = psS.tile([128, 1024], F32, tag="s", name="warm")
                nc.tensor.matmul(wps[:, 0:512], lhsT=wdum[:, 0:128],
                                 rhs=wdum, start=True, stop=True)
            # masks for h0 (both halves' worth) + h1 prefetch happens in-loop
            for kt in range(KT):
                if ROUTE[0][kt] == 'A':
                    fetch_mask(0, kt, 0)
                else:
                    fetch_mask(0, kt)
            qk_unit(1, 0, 0, 0, act_copy=True)()     # kT[hp0] keys 0:512
            qk_unit(0, 0, 0, 0, act_copy=True)()     # qT[hp0] halfA cols
            qk_unit(0, 0, 0, 512, act_copy=True)()
            qk_unit(1, 0, 0, 512, act_copy=True)()
            qk_unit(1, 0, 1, 0, act_copy=True)()
            qk_unit(1, 0, 1, 512, act_copy=True)()
            for i in range(4):
                v_unit(i)()

            # ---- main loop ----
            for gi in range(8):
                h, half = GORDER[gi]
                hp, ho = h // 2, (h % 2) * 64
                q0 = half * HALF
                for kt in range(KT):
                    # prefetch masks ~one half ahead
                    if gi < 7:
                        hn, halfn = GORDER[gi + 1]
                        rn = ROUTE[hn][kt]
                        if rn == 'A':
                            fetch_mask(hn, kt, halfn)
                        elif (hn, kt) not in mtiles:
                            fetch_mask(hn, kt)
                    # scores
                    s = psS.tile([128, 1024], F32, tag="s", name="s")
                    for s0 in (0, 512):
                        nc.tensor.matmul(
                            s[:, s0:s0 + 512],
                            lhsT=kTb[hp][ho:ho + 64, kt * 128:(kt + 1) * 128],
                            rhs=qTb[hp][ho:ho + 64, q0 + s0:q0 + s0 + 512],
                            start=True, stop=True)
                    # PV filler for previous g, emitted BEFORE the scores so
                    # the in-order PE stream hides the psS ring-buffer wait
                    # behind the PV group's work. Q0 groups on odd slots
                    # 1..7, Q1 on 9..12 so the finisher chain has slack
                    # before the next g reuses the single pvT buffer.
                    if gi >= 1:
                        if kt in (1, 3, 5, 7):
                            emit_pv_group(gi - 1, kt // 2)
                        elif kt in (9, 10, 11, 12):
                            emit_pv_group(gi - 1, kt - 5)
                        if kt == 8:
                            emit_finisher(gi - 1, 0)
                        elif kt == 13:
                            emit_finisher(gi - 1, 1)
                            note_finished(gi - 1)
                    for unit in FILL[gi].get(kt, ()):
                        unit()
                    # exp + mask
                    r = ROUTE[h][kt]
                    pt = pprob.tile([128, 1024], BF16, tag="pt", name="pt")
                    if r == 'F':
                        sA = psarg.tile([128, 1024], BF16, tag="sA", name="sA")
                        with tc.high_priority(offset=160):
                            nc.vector.tensor_tensor(
                                sA, s, mtiles[(h, kt)][:, q0:q0 + 1024],
                                mybir.AluOpType.add)
                            nc.vector.tensor_scalar(
                                pt.bitcast(I16), sA, A16 * SCALE, B16,
                                mybir.AluOpType.mult, mybir.AluOpType.add)
                    else:
                        nc.scalar.activation(pt, s, Exp, scale=SCALE)
                        if r == 'A':
                            nc.vector.tensor_mul(pt, pt, mtiles[(h, kt, half)])
                        else:
                            nc.gpsimd.tensor_mul(
                                pt, pt, mtiles[(h, kt)][:, q0:q0 + 1024])
                    pt_of[(gi, kt)] = pt
                    # half-A out-projections ride g7
                    if gi == 7 and kt >= 4 and kt % 2 == 0:
                        nt = (kt - 4) // 2
                        out_proj(nt, 'act' if nt % 2 == 0 else 'dve')

            # ---- drain: PV(g7) + finishers + remaining out-projs ----
            # g7 = h3B -> (hp1, halfB); h2B's quarters finished during g7, so
            # transpose each quarter as soon as h3B's finisher lands and
            # pipeline the halfB out-projections against the remaining PV.
            for qc in range(8):
                emit_pv_group(7, qc)
                if qc == 3:
                    emit_finisher(7, 0)
                    emit_transpose(1, 1, q4=0)
                if qc in (4, 5, 6, 7):
                    nt = 6 + (qc - 4)  # nt 6..9 (halfA tail + halfB q0..1)
                    if nt < 8:
                        out_proj(nt, 'dve' if nt % 2 == 0 else 'act')
                    elif nt == 8:
                        out_proj(8, 'act')
                    else:
                        out_proj(9, 'dve')
            emit_finisher(7, 1)
            emit_transpose(1, 1, q4=1)
            for nt in range(10, 16):
                out_proj(nt, 'act' if nt % 2 == 0 else 'dve')


def build_bass():
    nc = bacc.Bacc("TRN2", target_bir_lowering=False, debug=False, num_devices=8)
    with tile.TileContext(nc) as tc:
        nc.tc = tc
        build_graph(nc)
    nc.compile()
    return nc


def shard_inputs(x, drop_mask, w_qkv, w_out):
    """Host-side sharding: returns in_maps for the 8 cores."""
    x = np.asarray(x, dtype=np.float32)
    drop_mask = np.asarray(drop_mask)
    w_qkv = np.asarray(w_qkv, dtype=np.float32)
    w_out = np.asarray(w_out, dtype=np.float32)
    inner = HEADS * DH
    in_maps = []
    for c in range(8):
        b, g = c // GROUPS, c % GROUPS
        cols = slice(g * HL * DH, (g + 1) * HL * DH)
        wq = w_qkv[:, cols]
        wk = w_qkv[:, inner:][:, cols]
        wv = w_qkv[:, 2 * inner:][:, cols]
        nmb = np.zeros((max(NA, 1), 2, 128, HALF), dtype=NP_BF16)
        nmu = np.zeros((max(NP_, 1), 128, N), dtype=np.uint8)
        nmf = np.zeros((max(NF, 1), 128, N), dtype=NP_FP8E5)
        for h in range(HL):
            dmT = drop_mask[b, g * HL + h].T  # [keys, queries]
            for kt in range(KT):
                blk = dmT[kt * 128:(kt + 1) * 128, :]
                r = ROUTE[h][kt]
                if r == 'A':
                    nm = (~blk).astype(NP_BF16)
                    i = _IDX['A'][(h, kt)]
                    nmb[i, 0] = nm[:, :HALF]
                    nmb[i, 1] = nm[:, HALF:]
                elif r == 'P':
                    nmu[_IDX['P'][(h, kt)]] = (~blk).astype(np.uint8)
                else:
                    nmf[_IDX['F'][(h, kt)]] = np.where(
                        blk, np.float32(MADD), np.float32(0.0)).astype(NP_FP8E5)
        in_maps.append({
            "xT": np.ascontiguousarray(x[b].T).astype(NP_BF16),
            "wqkv": np.ascontiguousarray(
                np.concatenate([wq, wk, wv], axis=1)).astype(NP_BF16),
            "wout": np.ascontiguousarray(
                w_out[g * HL * DH:(g + 1) * HL * DH, :]).astype(NP_BF16),
            "nmb": nmb, "nmu": nmu, "nmf": nmf,
        })
    return in_maps


_CACHED_NC = None


def _get_nc():
    global _CACHED_NC
    if _CACHED_NC is None:
        _CACHED_NC = build_bass()
    return _CACHED_NC


def kernel(x, drop_mask, w_qkv, w_out, _trace=False):
    nc = _get_nc()
    in_maps = shard_inputs(x, drop_mask, w_qkv, w_out)
    res = run_bass_kernel_spmd(nc, in_maps, core_ids=list(range(8)), trace=_trace)
    outs = [np.asarray(r["out"], dtype=np.float32) for r in res.results]
    full = np.empty((B, N, D), dtype=np.float32)
    for b in range(B):
        full[b] = outs[b * GROUPS]
        for g in range(1, GROUPS):
            full[b] += outs[b * GROUPS + g]
    kernel.last_results = res
    return full


# revision 51
# speedup vs baseline: 1.0040x; 1.0040x over previous
"""Trainium2 Bass kernel for nn_Attention_64063732187236 (v4).

Reference computation (B=4, N=2048, DIM=512, HEADS=8, DIM_HEAD=64):
    qkv = x @ w_qkv ; q,k,v = split(qkv) -> [b,h,n,64]
    atten = softmax((q * HEADS**-0.5) @ k^T + drop_mask * -1e12)
    out   = (atten @ v) reshaped -> [b,n,512] @ w_out

Sharding: 8 cores = 4 batches x 2 head-groups (4 heads each); host sums the
two head-group partial outputs per batch.

v4 design (cost-model driven; ~110us/core PE floor):
- Transposed PV: out[q,d] psum tiles via lhsT=pt[128keys,128q] slices and
  rhs=v[128,65] -> PV costs 65 cycles/matmul (27.7us total vs 54.6 in the
  [d,q] orientation), and the rowsum column lands per-partition so softmax
  normalization folds into the psum->SBUF copy as a per-partition
  broadcast multiply (free).
- The normalized pvq [128q, (qc, head-parity, d)] tile is transposed back
  to attnT [2*64 dims, qc, 128 q] by ONE DMA-engine xbar transpose per
  (pair, half) (out[p,t,c] = in[c, t*128+p]), so no PE transpose and no
  extra copies; out-projection reads attnT directly (128-contraction).
- exp split three ways to balance ACT/DVE/Pool below the PE floor:
  * ACT route: exact exp from psum, then mask multiply {0,1}:
    - on DVE (bf16 mask, 2x rate), or
    - on Pool/gpsimd (u8 mask -- 1-byte DMA, dtype-blind Q7 rate).
  * DVE fused route: tensor_tensor add of an fp8e5 additive mask
    {0, -1024} onto the psum scores (bf16 out), then a 4x-rate
    tensor_scalar Schraudolph exp (bitcast-i16); masked entries drive the
    i16 round to saturation = 0x8000 = -0.0 exactly.
- PV(g) runs one half behind scores(g) (query-chunk-outer so psum
  accumulation groups stay sequential per bank); g-order
  h0A h0B h1A h1B h2A h3A h2B h3B lets half-A out-projections ride g7.
- PSUM: psS 2x[128,1024]f32 (4 banks) + pvT 2x[128,4,2,64] (2) +
  rs [128,512] (1) + po [128,512] (1) = 8 banks exactly.
"""

import numpy as np

import concourse.bass as bass
import concourse.bacc as bacc
import concourse.tile as tile
from concourse import mybir
from concourse.bass_utils import run_bass_kernel_spmd

F32 = mybir.dt.float32
BF16 = mybir.dt.bfloat16
U8 = mybir.dt.uint8
FP8E5 = mybir.dt.float8e5
I16 = mybir.dt.int16
NP_BF16 = mybir.dt.np(BF16)
NP_FP8E5 = mybir.dt.np(FP8E5)

# Full-size problem constants
B, N, D = 4, 2048, 512
HEADS, DH = 8, 64
HL = 4                # heads per core (local)
GROUPS = HEADS // HL  # head groups = 2
HP = HL // 2          # head pairs per core
KT = N // 128         # key tiles
HALF = N // 2
SCALE = float(HEADS) ** -0.5   # reference quirk: scales by heads, not dim_head
Exp = mybir.ActivationFunctionType.Exp
A16 = 128.0 / float(np.log(2.0))
B16 = 127.0 * 128 - 8
MADD = -1024.0  # additive mask value (exact in fp8e5); saturates TS to -0.0

# g order: (h, half) so that both heads of a pair finish PV close together
# and half-A out-projections can ride g7.
GORDER = [(0, 0), (0, 1), (1, 0), (1, 1), (2, 0), (3, 0), (2, 1), (3, 1)]

# Route per (h, kt) unit, applied to both halves:
#   'A' = ACT exp + DVE bf16 mask-mul
#   'P' = ACT exp + Pool u8 mask-mul
#   'F' = DVE fused fp8e5 additive mask + 4x Schraudolph
# Per head: 16 kt -> mix tuned so ACT ~100 tiles, DVE-fused 28, Pool 48.
_PAT = ['P', 'F', 'P', 'A', 'P', 'F', 'P', 'A', 'P', 'F', 'A', 'A', 'F', 'P', 'A', 'A']
_PAT3 = ['P', 'F', 'P', 'A', 'P', 'F', 'P', 'A', 'P', 'F', 'A', 'A', 'F', 'P', 'A', 'A']
ROUTE = {h: list(_PAT if h < 3 else _PAT3) for h in range(HL)}
# index of each (h, kt) unit within its dtype-packed host tensor
_IDX = {'A': {}, 'P': {}, 'F': {}}
for _h in range(HL):
    for _kt in range(KT):
        r = ROUTE[_h][_kt]
        _IDX[r][(_h, _kt)] = len(_IDX[r])
NA, NP_, NF = len(_IDX['A']), len(_IDX['P']), len(_IDX['F'])


def build_graph(nc):
    tc = nc.tc

    xT = nc.dram_tensor("xT", [D, N], BF16, kind="ExternalInput").ap()
    wqkv = nc.dram_tensor("wqkv", [D, 3 * HL * DH], BF16, kind="ExternalInput").ap()
    wout = nc.dram_tensor("wout", [HL * DH, D], BF16, kind="ExternalInput").ap()
    # A-route masks packed per half: [unit, half, 128 keys, 1024 queries]
    nmb = nc.dram_tensor("nmb", [max(NA, 1), 2, 128, HALF], BF16,
                         kind="ExternalInput").ap()
    nmu = nc.dram_tensor("nmu", [max(NP_, 1), 128, N], U8,
                         kind="ExternalInput").ap()
    nmf = nc.dram_tensor("nmf", [max(NF, 1), 128, N], FP8E5,
                         kind="ExternalInput").ap()
    out = nc.dram_tensor("out", [N, D], BF16, kind="ExternalOutput").ap()

    with tc.tile_pool(name="wts", bufs=1) as wts, \
         tc.tile_pool(name="persist", bufs=1) as persist, \
         tc.tile_pool(name="pmb", bufs=14) as pmb, \
         tc.tile_pool(name="pmu", bufs=12) as pmu, \
         tc.tile_pool(name="pmf", bufs=8) as pmf, \
         tc.tile_pool(name="pprob", bufs=33) as pprob, \
         tc.tile_pool(name="psarg", bufs=2) as psarg, \
         tc.tile_pool(name="pout", bufs=4) as pout:

        # ---- weights / inputs ----
        wb = [wts.tile([128, 3 * HL * DH], BF16, tag=f"wb{dc}", name=f"wb{dc}")
              for dc in range(4)]
        xTb = [wts.tile([128, N], BF16, tag=f"xtb{dc}", name=f"xtb{dc}")
               for dc in range(4)]
        woutb = [wts.tile([128, D], BF16, tag=f"wob{c}", name=f"wob{c}")
                 for c in range(HP)]
        wdum = wts.tile([128, 512], BF16, tag="wdum", name="wdum")
        nc.vector.memset(wdum, 0.0)

        for dc in range(4):
            nc.sync.dma_start(out=wb[dc], in_=wqkv[dc * 128:(dc + 1) * 128, :])
            nc.sync.dma_start(out=xTb[dc][:, 0:512],
                              in_=xT[dc * 128:(dc + 1) * 128, 0:512])
            nc.scalar.dma_start(out=xTb[dc][:, 512:HALF],
                                in_=xT[dc * 128:(dc + 1) * 128, 512:HALF])
        for c in range(HP):
            nc.sync.dma_start(out=woutb[c], in_=wout[c * 128:(c + 1) * 128, :])
        for dc in range(4):
            nc.scalar.dma_start(out=xTb[dc][:, HALF:N],
                                in_=xT[dc * 128:(dc + 1) * 128, HALF:N])

        # ---- persistent tiles ----
        qTb = [persist.tile([128, N], BF16, tag=f"qT{p}", name=f"qT{p}")
               for p in range(HP)]
        kTb = [persist.tile([128, N], BF16, tag=f"kT{p}", name=f"kT{p}")
               for p in range(HP)]
        vplus = [persist.tile([128, HL, DH + 1], BF16, tag=f"vp{t}",
                              name=f"vp{t}") for t in range(KT)]
        # pvq[(hp, half)]: [128 q, qc, head-parity, d] normalized bf16
        pvq = {}
        attnT = {}
        rinv = {}
        for hp in range(HP):
            for half in range(2):
                pvq[(hp, half)] = persist.tile(
                    [128, 8, 2, DH], BF16, tag=f"pvq{hp}{half}",
                    name=f"pvq{hp}{half}")
                attnT[(hp, half)] = persist.tile(
                    [128, 8, 128], BF16, tag=f"at{hp}{half}",
                    name=f"at{hp}{half}")
                rinv[(hp, half)] = persist.tile(
                    [128, 8, 2], F32, tag=f"ri{hp}{half}", name=f"ri{hp}{half}")

        # mask tiles per (h, kt) unit (P/F routes) or per (h, kt, half) (A)
        mtiles = {}

        def fetch_mask(h, kt, half=None):
            r = ROUTE[h][kt]
            if r == 'A':
                t = pmb.tile([128, HALF], BF16, tag="nmA", name=f"ma{h}_{kt}_{half}")
                nc.sync.dma_start(out=t, in_=nmb[_IDX['A'][(h, kt)], half])
                mtiles[(h, kt, half)] = t
            elif r == 'P':
                t = pmu.tile([128, N], U8, tag="nmP", name=f"mp{h}_{kt}")
                nc.sync.dma_start(out=t, in_=nmu[_IDX['P'][(h, kt)]])
                mtiles[(h, kt)] = t
            else:
                t = pmf.tile([128, N], FP8E5, tag="nmF", name=f"mf{h}_{kt}")
                nc.sync.dma_start(out=t, in_=nmf[_IDX['F'][(h, kt)]])
                mtiles[(h, kt)] = t

        # pt tiles per (g, kt): [128 keys, 1024 queries] bf16
        pt_of = {}

        with tc.tile_pool(name="psS", bufs=3, space="PSUM") as psS, \
             tc.tile_pool(name="psPV", bufs=1, space="PSUM") as psPV, \
             tc.tile_pool(name="psRS", bufs=1, space="PSUM") as psRS:

            # ---- projection filler units ----
            def qk_unit(which, hp, half, s0, act_copy=False):
                def emit():
                    off = which * HL * DH
                    col = half * HALF + s0
                    ps = psS.tile([128, 1024], F32, tag="s", name="psqk")
                    for dc in range(4):
                        nc.tensor.matmul(
                            ps[:, 0:512],
                            lhsT=wb[dc][:, off + hp * 128: off + (hp + 1) * 128],
                            rhs=xTb[dc][:, col:col + 512],
                            start=(dc == 0), stop=(dc == 3))
                    dst = qTb if which == 0 else kTb
                    if act_copy:
                        nc.scalar.copy(dst[hp][:, col:col + 512], ps[:, 0:512])
                    else:
                        with tc.high_priority(offset=110):
                            nc.vector.tensor_copy(dst[hp][:, col:col + 512],
                                                  ps[:, 0:512])
                return emit

            def v_unit(nt):
                def emit():
                    voff = 2 * HL * DH
                    ps = psS.tile([128, 1024], F32, tag="s", name="psv")
                    for dc in range(4):
                        nc.tensor.matmul(
                            ps[:, 0:HL * DH],
                            lhsT=xTb[dc][:, nt * 128:(nt + 1) * 128],
                            rhs=wb[dc][:, voff: voff + HL * DH],
                            start=(dc == 0), stop=(dc == 3))
                    nc.vector.memset(vplus[nt], 1.0)
                    with tc.high_priority(offset=110):
                        nc.vector.tensor_copy(
                            vplus[nt][:, :, 0:DH],
                            ps[:, 0:HL * DH].rearrange("p (h d) -> p h d", h=HL))
                return emit

            # filler schedule: {g: {kt: [unit...]}}
            FILL = {g: {} for g in range(8)}

            def put(g, kt, unit):
                FILL[g].setdefault(kt, []).append(unit)

            # g0 fillers: v4..v15 (v0-3 upfront), qT[hp0] halfB
            for i in range(4, KT):
                put(0, i - 4, v_unit(i))
            put(0, 12, qk_unit(0, 0, 1, 0))
            put(0, 14, qk_unit(0, 0, 1, 512))
            # g1-g3 fillers: kT[hp1] (4 units), qT[hp1] halfA (2), halfB (2)
            put(1, 2, qk_unit(1, 1, 0, 0))
            put(1, 6, qk_unit(1, 1, 0, 512))
            put(1, 10, qk_unit(1, 1, 1, 0))
            put(1, 14, qk_unit(1, 1, 1, 512))
            put(2, 2, qk_unit(0, 1, 0, 0))
            put(2, 8, qk_unit(0, 1, 0, 512))
            put(3, 2, qk_unit(0, 1, 1, 0))
            put(3, 8, qk_unit(0, 1, 1, 512))

            # ---- PV one half behind: per g, 8 qc groups (2 quarters) ----
            def emit_pv_group(gi, qc):
                """Query-chunk group qc (0..7) of PV for g=GORDER[gi]."""
                h, half = GORDER[gi]
                hp, hpar = h // 2, h % 2
                q4, qc_l = qc // 4, qc % 4
                key = (gi, q4)
                if key not in emit_pv_group.tiles:
                    emit_pv_group.tiles[key] = psPV.tile(
                        [128, 4, 2, DH], F32, tag="pv", name=f"pv{gi}_{q4}")
                pvT = emit_pv_group.tiles[key]
                if (gi,) not in emit_pv_group.rs:
                    emit_pv_group.rs[(gi,)] = psRS.tile(
                        [128, 512], F32, tag="rs", name=f"rs{gi}")
                rs = emit_pv_group.rs[(gi,)]
                col = qc * 2 + hpar
                # P-route tiles (Pool-masked, latest to complete) last
                kts = ([kt for kt in range(KT) if ROUTE[h][kt] != 'P']
                       + [kt for kt in range(KT) if ROUTE[h][kt] == 'P'])
                for i, kt in enumerate(kts):
                    pt = pt_of[(gi, kt)]
                    lhsT = pt[:, qc * 128:(qc + 1) * 128]
                    nc.tensor.matmul(
                        pvT[:, qc_l, hpar, :], lhsT=lhsT,
                        rhs=vplus[kt][:, h, 0:DH],
                        start=(i == 0), stop=(i == KT - 1))
                    nc.tensor.matmul(
                        rs[:, col:col + 1], lhsT=lhsT,
                        rhs=vplus[kt][:, h, DH:DH + 1],
                        start=(i == 0), stop=(i == KT - 1))

            emit_pv_group.tiles = {}
            emit_pv_group.rs = {}

            def emit_finisher(gi, q4):
                """Normalize quarter q4 of g=GORDER[gi] into pvq."""
                h, half = GORDER[gi]
                hp, hpar = h // 2, h % 2
                pvT = emit_pv_group.tiles.pop((gi, q4))
                rs = emit_pv_group.rs[(gi,)]
                rs16 = rs[:, 0:16].rearrange("p (a b) -> p a b", a=8)
                ri = rinv[(hp, half)]
                with tc.high_priority(offset=140):
                    nc.vector.reciprocal(ri[:, q4 * 4:(q4 + 1) * 4, hpar],
                                         rs16[:, q4 * 4:(q4 + 1) * 4, hpar])
                    nc.vector.tensor_tensor(
                        pvq[(hp, half)][:, q4 * 4:(q4 + 1) * 4, hpar, :],
                        pvT[:, :, hpar, :],
                        ri[:, q4 * 4:(q4 + 1) * 4, hpar].unsqueeze(2)
                          .broadcast_to([128, 4, DH]),
                        mybir.AluOpType.mult)

            def emit_transpose(hp, half, q4=None):
                if q4 is None:
                    nc.sync.dma_start(
                        out=attnT[(hp, half)],
                        in_=pvq[(hp, half)].rearrange("p a b d -> p (a b d)"),
                        transpose=True)
                else:
                    nc.sync.dma_start(
                        out=attnT[(hp, half)][:, q4 * 4:(q4 + 1) * 4, :],
                        in_=pvq[(hp, half)][:, q4 * 4:(q4 + 1) * 4]
                        .rearrange("p a b d -> p (a b d)"),
                        transpose=True)

            def out_proj(nt, copy_eng):
                half, qcl = nt // 8, nt % 8
                po = psS.tile([128, 1024], F32, tag="s", name="po")[:, 0:D]
                for c in range(HP):
                    nc.tensor.matmul(
                        po, lhsT=attnT[(c, half)][:, qcl, :],
                        rhs=woutb[c], start=(c == 0), stop=(c == HP - 1))
                ob = pout.tile([128, D], BF16, tag="ob", name="ob")
                if copy_eng == 'act':
                    nc.scalar.copy(ob, po)
                else:
                    nc.vector.tensor_copy(ob, po)
                nc.sync.dma_start(out=out[nt * 128:(nt + 1) * 128, :], in_=ob)

            # transposes done per (hp, half) once both heads' finishers ran
            fin_count = {}

            def note_finished(gi):
                h, half = GORDER[gi]
                hp = h // 2
                fin_count[(hp, half)] = fin_count.get((hp, half), 0) + 1
                if fin_count[(hp, half)] == 2:
                    emit_transpose(hp, half)

            # ---- PE warm-up + upfront projections ----
            for _ in range(5):
                wps = psS.tile([128, 1024], F32, tag="s", name="warm")
                nc.tensor.matmul(wps[:, 0:512], lhsT=wdum[:, 0:128],
                                 rhs=wdum, start=True, stop=True)
            # masks for h0 (both halves' worth) + h1 prefetch happens in-loop
            for kt in range(KT):
                if ROUTE[0][kt] == 'A':
                    fetch_mask(0, kt, 0)
                else:
                    fetch_mask(0, kt)
            qk_unit(1, 0, 0, 0, act_copy=True)()     # kT[hp0] keys 0:512
            qk_unit(0, 0, 0, 0, act_copy=True)()     # qT[hp0] halfA cols
            qk_unit(0, 0, 0, 512, act_copy=True)()
            qk_unit(1, 0, 0, 512, act_copy=True)()
            qk_unit(1, 0, 1, 0, act_copy=True)()
            qk_unit(1, 0, 1, 512, act_copy=True)()
            for i in range(4):
                v_unit(i)()

            # ---- main loop ----
            for gi in range(8):
                h, half = GORDER[gi]
                hp, ho = h // 2, (h % 2) * 64
                q0 = half * HALF
                for kt in range(KT):
                    # prefetch masks ~one half ahead
                    if gi < 7:
                        hn, halfn = GORDER[gi + 1]
                        rn = ROUTE[hn][kt]
                        if rn == 'A':
                            fetch_mask(hn, kt, halfn)
                        elif (hn, kt) not in mtiles:
                            fetch_mask(hn, kt)
                    # scores
                    s = psS.tile([128, 1024], F32, tag="s", name="s")
                    for s0 in (0, 512):
                        nc.tensor.matmul(
                            s[:, s0:s0 + 512],
                            lhsT=kTb[hp][ho:ho + 64, kt * 128:(kt + 1) * 128],
                            rhs=qTb[hp][ho:ho + 64, q0 + s0:q0 + s0 + 512],
                            start=True, stop=True)
                    # PV filler for previous g, emitted BEFORE the scores so
                    # the in-order PE stream hides the psS ring-buffer wait
                    # behind the PV group's work. Q0 groups on odd slots
                    # 1..7, Q1 on 9..12 so the finisher chain has slack
                    # before the next g reuses the single pvT buffer.
                    if gi >= 1:
                        if kt in (1, 3, 5, 7):
                            emit_pv_group(gi - 1, kt // 2)
                        elif kt in (9, 10, 11, 12):
                            emit_pv_group(gi - 1, kt - 5)
                        if kt == 8:
                            emit_finisher(gi - 1, 0)
                        elif kt == 13:
                            emit_finisher(gi - 1, 1)
                            note_finished(gi - 1)
                    for unit in FILL[gi].get(kt, ()):
                        unit()
                    # exp + mask
                    r = ROUTE[h][kt]
                    pt = pprob.tile([128, 1024], BF16, tag="pt", name="pt")
                    if r == 'F':
                        sA = psarg.tile([128, 1024], BF16, tag="sA", name="sA")
                        with tc.high_priority(offset=160):
                            nc.vector.tensor_tensor(
                                sA, s, mtiles[(h, kt)][:, q0:q0 + 1024],
                                mybir.AluOpType.add)
                            nc.vector.tensor_scalar(
                                pt.bitcast(I16), sA, A16 * SCALE, B16,
                                mybir.AluOpType.mult, mybir.AluOpType.add)
                    else:
                        nc.scalar.activation(pt, s, Exp, scale=SCALE)
                        if r == 'A':
                            nc.vector.tensor_mul(pt, pt, mtiles[(h, kt, half)])
                        else:
                            nc.gpsimd.tensor_mul(
                                pt, pt, mtiles[(h, kt)][:, q0:q0 + 1024])
                    pt_of[(gi, kt)] = pt
                    # half-A out-projections ride g7
                    if gi == 7 and kt >= 4 and kt % 2 == 0:
                        nt = (kt - 4) // 2
                        out_proj(nt, 'act' if nt % 2 == 0 else 'dve')

            # ---- drain: PV(g7) + finishers + remaining out-projs ----
            # g7 = h3B -> (hp1, halfB); h2B's quarters finished during g7, so
            # transpose each quarter as soon as h3B's finisher lands and
            # pipeline the halfB out-projections against the remaining PV.
            for qc in range(8):
                emit_pv_group(7, qc)
                if qc == 3:
                    emit_finisher(7, 0)
                    emit_transpose(1, 1, q4=0)
                if qc in (4, 5, 6, 7):
                    nt = 6 + (qc - 4)  # nt 6..9 (halfA tail + halfB q0..1)
                    if nt < 8:
                        out_proj(nt, 'dve' if nt % 2 == 0 else 'act')
                    elif nt == 8:
                        out_proj(8, 'act')
                    else:
                        out_proj(9, 'dve')
            emit_finisher(7, 1)
            emit_transpose(1, 1, q4=1)
            for nt in range(10, 16):
                out_proj(nt, 'act' if nt % 2 == 0 else 'dve')


def build_bass():
    nc = bacc.Bacc("TRN2", target_bir_lowering=False, debug=False, num_devices=8)
    with tile.TileContext(nc) as tc:
        nc.tc = tc
        build_graph(nc)
    nc.compile()
    return nc


def shard_inputs(x, drop_mask, w_qkv, w_out):
    """Host-side sharding: returns in_maps for the 8 cores."""
    x = np.asarray(x, dtype=np.float32)
    drop_mask = np.asarray(drop_mask)
    w_qkv = np.asarray(w_qkv, dtype=np.float32)
    w_out = np.asarray(w_out, dtype=np.float32)
    inner = HEADS * DH
    in_maps = []
    for c in range(8):
        b, g = c // GROUPS, c % GROUPS
        cols = slice(g * HL * DH, (g + 1) * HL * DH)
        wq = w_qkv[:, cols]
        wk = w_qkv[:, inner:][:, cols]
        wv = w_qkv[:, 2 * inner:][:, cols]
        nmb = np.zeros((max(NA, 1), 2, 128, HALF), dtype=NP_BF16)
        nmu = np.zeros((max(NP_, 1), 128, N), dtype=np.uint8)
        nmf = np.zeros((max(NF, 1), 128, N), dtype=NP_FP8E5)
        for h in range(HL):
            dmT = drop_mask[b, g * HL + h].T  # [keys, queries]
            for kt in range(KT):
                blk = dmT[kt * 128:(kt + 1) * 128, :]
                r = ROUTE[h][kt]
                if r == 'A':
                    nm = (~blk).astype(NP_BF16)
                    i = _IDX['A'][(h, kt)]
                    nmb[i, 0] = nm[:, :HALF]
                    nmb[i, 1] = nm[:, HALF:]
                elif r == 'P':
                    nmu[_IDX['P'][(h, kt)]] = (~blk).astype(np.uint8)
                else:
                    nmf[_IDX['F'][(h, kt)]] = np.where(
                        blk, np.float32(MADD), np.float32(0.0)).astype(NP_FP8E5)
        in_maps.append({
            "xT": np.ascontiguousarray(x[b].T).astype(NP_BF16),
            "wqkv": np.ascontiguousarray(
                np.concatenate([wq, wk, wv], axis=1)).astype(NP_BF16),
            "wout": np.ascontiguousarray(
                w_out[g * HL * DH:(g + 1) * HL * DH, :]).astype(NP_BF16),
            "nmb": nmb, "nmu": nmu, "nmf": nmf,
        })
    return in_maps


_CACHED_NC = None


def _get_nc():
    global _CACHED_NC
    if _CACHED_NC is None:
        _CACHED_NC = build_bass()
    return _CACHED_NC


def kernel(x, drop_mask, w_qkv, w_out, _trace=False):
    nc = _get_nc()
    in_maps = shard_inputs(x, drop_mask, w_qkv, w_out)
    res = run_bass_kernel_spmd(nc, in_maps, core_ids=list(range(8)), trace=_trace)
    outs = [np.asarray(r["out"], dtype=np.float32) for r in res.results]
    full = np.empty((B, N, D), dtype=np.float32)
    for b in range(B):
        full[b] = outs[b * GROUPS]
        for g in range(1, GROUPS):
            full[b] += outs[b * GROUPS + g]
    kernel.last_results = res
    return full
